# revision 1
# baseline (speedup 1.0000x reference)
"""Self-contained TRN2 Bass kernel for nn_MultiHeadAttention_77833397338481.

kernel(**inputs) takes the FULL unsharded inputs (Q, K, V [2,1024,1024],
Wq/Wk/Wv/Wo [1024,1024], biases [1024]) and returns the FULL output
[2, 1024, 1024]. 8 NeuronCores = batch(2) x head-group(4); fp32r matmuls,
row-tiled scores^T, fused-scale exp, ones-column softmax denominators,
per-core partial output projection summed on the host.
"""

import numpy as np

import concourse.bacc as bacc
import concourse.mybir as mybir
import concourse.tile as tile

F32 = mybir.dt.float32
F32R = mybir.dt.float32r
AF = mybir.ActivationFunctionType

D = 1024
S = 1024
B = 2
E = 16
NHQ = 4
NQUAD = 4
HPC = NHQ * NQUAD
VW = HPC * 17  # 272
SCALE = 1.0 / 32.0
ND = D // 128
NS = S // 128


def round_fp32r(x):
    u = np.ascontiguousarray(x, np.float32).view(np.uint32)
    r = ((u.astype(np.uint64) + 0x800) & 0xFFFFF000).astype(np.uint32)
    return r.view(np.float32)


def build_nc(phases=3):
    nc = bacc.Bacc("TRN2", target_bir_lowering=False, debug=False, num_devices=8)

    xt_q = nc.dram_tensor("xt_q", [D, S], F32R, kind="ExternalInput")
    xt_k = nc.dram_tensor("xt_k", [D, S], F32R, kind="ExternalInput")
    xt_v = nc.dram_tensor("xt_v", [D, S], F32R, kind="ExternalInput")
    wqt = nc.dram_tensor("wqt", [D, 512], F32R, kind="ExternalInput")
    wkt = nc.dram_tensor("wkt", [D, 512], F32R, kind="ExternalInput")
    wvt = nc.dram_tensor("wvt", [D, VW], F32R, kind="ExternalInput")
    wot = nc.dram_tensor("wot", [512, D], F32R, kind="ExternalInput")
    ind = nc.dram_tensor("ind", [NHQ, 128], F32R, kind="ExternalInput")
    bvrow = nc.dram_tensor("bvrow", [1, VW], F32, kind="ExternalInput")
    bqp = nc.dram_tensor("bqp", [128, NQUAD], F32, kind="ExternalInput")
    bkp = nc.dram_tensor("bkp", [128, NQUAD], F32, kind="ExternalInput")
    out_d = nc.dram_tensor("out_part", [S, D], F32, kind="ExternalOutput")

    with tile.TileContext(nc) as tc:
        with (
            tc.tile_pool(name="persist", bufs=1) as pp,
            tc.tile_pool(name="proj", bufs=1) as jp,
            tc.tile_pool(name="attn", bufs=1) as ap_,
            tc.tile_pool(name="psum", space="PSUM", bufs=1) as ps,
        ):
            # --- warm up the exp table ASAP ---
            dummy = pp.tile([1, 8], F32, name="dummy")
            nc.vector.memset(dummy, 0.0)
            dummy2 = pp.tile([1, 8], F32, name="dummy2")
            nc.scalar.activation(dummy2, dummy, AF.Exp)

            # --- constants ---
            ind_sb = pp.tile([NHQ, 128], F32R, name="ind_sb")
            nc.sync.dma_start(out=ind_sb, in_=ind[:])
            ones1 = pp.tile([1, 128], F32, name="ones1")
            nc.vector.memset(ones1, 1.0)
            bvrow_sb = pp.tile([1, VW], F32, name="bvrow_sb")
            nc.sync.dma_start(out=bvrow_sb, in_=bvrow[:])
            bq_sb = pp.tile([128, NQUAD], F32, name="bq_sb")
            nc.sync.dma_start(out=bq_sb, in_=bqp[:])
            bk_sb = pp.tile([128, NQUAD], F32, name="bk_sb")
            nc.sync.dma_start(out=bk_sb, in_=bkp[:])

            biasB_ps = ps.tile([128, VW], F32, name="biasB_ps", tag="ctx", bufs=4)
            nc.tensor.matmul(biasB_ps, ones1, bvrow_sb, start=True, stop=True)
            biasB = pp.tile([128, VW], F32, name="biasB")
            nc.vector.tensor_copy(biasB, biasB_ps)

            # --- persistent activations ---
            qt_sb = [pp.tile([128, S], F32R, name=f"qt{t}") for t in range(NQUAD)]
            kt_sb = [pp.tile([128, S], F32R, name=f"kt{t}") for t in range(NQUAD)]
            va_sb = [pp.tile([128, VW], F32R, name=f"va{s}") for s in range(NS)]
            ctxp = [pp.tile([128, S], F32R, name=f"ctxp{t}") for t in range(NQUAD)]
            z512 = jp.tile([128, 512], F32, name="z512", tag="z", bufs=1)
            nc.vector.memset(z512, 0.0)
            for t in range(NQUAD):
                nc.vector.tensor_copy(ctxp[t][:, 0:512], z512)
                nc.vector.tensor_copy(ctxp[t][:, 512:1024], z512)
            wot_sb = [
                ap_.tile([128, D], F32R, name=f"wot{t}", tag="wot", bufs=NQUAD)
                for t in range(NQUAD)
            ]

            # ============ projections (d-streamed, one xt pass each) ========
            def stream_proj(xt_dram, w_dram, wv_width, emit_mms, emit_evacs, name):
                """One pass over d: 8 accumulation groups (4 in the sc tile's
                512-wide slices + 4 in ctx-tag tiles)."""
                scg_a = ps.tile([128, 1024], F32, name=f"scga_{name}", tag="sc", bufs=2)
                scg_b = ps.tile([128, 1024], F32, name=f"scgb_{name}", tag="sc", bufs=2)
                scg = (scg_a, scg_b)
                ctxg = [
                    ps.tile([128, 512], F32, name=f"cg_{name}{g}", tag="ctx", bufs=4)
                    for g in range(4)
                ]
                xts, ws = [], []
                for d in range(ND):
                    xv = jp.tile(
                        [128, S], F32R, name=f"x_{name}{d}", tag="xt", bufs=8
                    )
                    nc.sync.dma_start(out=xv, in_=xt_dram[128 * d : 128 * (d + 1), :])
                    xts.append(xv)
                    wv = jp.tile(
                        [128, wv_width], F32R, name=f"w_{name}{d}", tag="wp", bufs=6
                    )
                    nc.sync.dma_start(out=wv, in_=w_dram[128 * d : 128 * (d + 1), :])
                    ws.append(wv)
                for d in range(ND):
                    emit_mms(d, xts[d], ws[d], scg, ctxg)
                emit_evacs(scg, ctxg)

            def v_mms(d, xv, wv, scg, ctxg):
                # group s: out va[s] [128, VW]; s<4 -> scg slice 512*s, else ctx
                for s in range(NS):
                    dst = (
                        scg[s // 2][:, 512 * (s % 2) : 512 * (s % 2) + VW]
                        if s < 4
                        else ctxg[s - 4][:, 0:VW]
                    )
                    nc.tensor.matmul(
                        dst,
                        xv[:, 128 * s : 128 * (s + 1)],
                        wv,
                        start=(d == 0),
                        stop=(d == ND - 1),
                    )

            def v_evacs(scg, ctxg):
                for s in range(NS):
                    src = (
                        scg[s // 2][:, 512 * (s % 2) : 512 * (s % 2) + VW]
                        if s < 4
                        else ctxg[s - 4][:, 0:VW]
                    )
                    nc.vector.tensor_add(va_sb[s], src, biasB)

            def qk_mms_factory(which):
                def mms(d, xv, wv, scg, ctxg):
                    for t in range(NQUAD):
                        for h in range(2):
                            g = 2 * t + h
                            dst = (
                                scg[g // 2][:, 512 * (g % 2) : 512 * (g % 2 + 1)]
                                if g < 4
                                else ctxg[g - 4]
                            )
                            nc.tensor.matmul(
                                dst,
                                wv[:, 128 * t : 128 * (t + 1)],
                                xv[:, 512 * h : 512 * (h + 1)],
                                start=(d == 0),
                                stop=(d == ND - 1),
                            )

                return mms

            def qk_evacs_factory(dst_tiles, bias):
                def evacs(scg, ctxg):
                    for t in range(NQUAD):
                        for h in range(2):
                            g = 2 * t + h
                            src = (
                                scg[g // 2][:, 512 * (g % 2) : 512 * (g % 2 + 1)]
                                if g < 4
                                else ctxg[g - 4]
                            )
                            nc.vector.tensor_scalar(
                                dst_tiles[t][:, 512 * h : 512 * (h + 1)],
                                src,
                                bias[:, t : t + 1],
                                None,
                                mybir.AluOpType.add,
                            )

                return evacs

            stream_proj(xt_v, wvt, VW, v_mms, v_evacs, "v")
            stream_proj(xt_q, wqt, 512, qk_mms_factory("q"), qk_evacs_factory(qt_sb, bq_sb), "q")
            stream_proj(xt_k, wkt, 512, qk_mms_factory("k"), qk_evacs_factory(kt_sb, bk_sb), "k")

            # --- PE warm-up burst: ~5us of dense matmuls so HAM unthrottles ---
            if False:
                wu = ps.tile([128, 2048], F32, name="wu", tag="sc", bufs=1)
                for w_i in range(24):
                    nc.tensor.matmul(
                        wu[:, 512 * (w_i % 4) : 512 * (w_i % 4 + 1)],
                        kt_sb[0][:, 0:128],
                        qt_sb[0][:, 0:512],
                        start=True,
                        stop=True,
                    )
                nc.vector.tensor_copy(dummy, wu[0:1, 0:8])

            for t in range(NQUAD):
                nc.sync.dma_start(out=wot_sb[t], in_=wot[128 * t : 128 * (t + 1), :])

            # ================= attention (n-outer) =================
            def outproj_group(m, dc):
                po = ps.tile([128, 512], F32, name=f"po{m}{dc}", tag="ctx", bufs=4)
                for t_ in range(NQUAD):
                    nc.tensor.matmul(
                        po,
                        ctxp[t_][:, 128 * m : 128 * (m + 1)],
                        wot_sb[t_][:, 512 * dc : 512 * (dc + 1)],
                        start=(t_ == 0),
                        stop=(t_ == NQUAD - 1),
                    )
                og = ap_.tile([128, 512], F32, name=f"og{m}{dc}", tag="og", bufs=4)
                nc.vector.tensor_copy(og, po)
                nc.sync.dma_start(
                    out=out_d[128 * m : 128 * (m + 1), 512 * dc : 512 * (dc + 1)],
                    in_=og,
                )

            for n in range(2 if phases >= 2 else 0):  # sq chunks of 512
                for t in range(NQUAD):
                    ctx_t = [
                        ps.tile([17, 512], F32, name=f"ctx{t}{n}{j}", tag="ctx", bufs=4)
                        for j in range(NHQ)
                    ]
                    ex_tiles = []
                    for i in range(NS):  # sk blocks of 128
                        sc_a = ps.tile(
                            [128, 1024], F32, name=f"sca{t}{n}{i}", tag="sc", bufs=2
                        )
                        sc_b = ps.tile(
                            [128, 1024], F32, name=f"scb{t}{n}{i}", tag="sc", bufs=2
                        )
                        ex = ap_.tile(
                            [128, 2048], F32R, name=f"ex{t}{n}{i}", tag="ex", bufs=5
                        )
                        for j in range(NHQ):
                            sch = sc_a if j < 2 else sc_b
                            nc.tensor.matmul(
                                sch[:, 512 * (j % 2) : 512 * (j % 2 + 1)],
                                kt_sb[t][32 * j : 32 * j + 32, 128 * i : 128 * (i + 1)],
                                qt_sb[t][32 * j : 32 * j + 32, 512 * n : 512 * (n + 1)],
                                start=True,
                                stop=True,
                                tile_position=(32 * j, 0),
                            )
                        nc.scalar.activation(ex[:, 0:1024], sc_a, AF.Exp, scale=SCALE)
                        nc.scalar.activation(
                            ex[:, 1024:2048], sc_b, AF.Exp, scale=SCALE
                        )
                        ex_tiles.append(ex)
                        if phases >= 3 and n == 1 and i in (3, 6):
                            _gi = 2 * t + (1 if i == 6 else 0)
                            outproj_group(_gi // 2, _gi % 2)
                        if i >= 1:
                            for j in range(NHQ):
                                mq = 17 * (NHQ * t + j)
                                nc.tensor.matmul(
                                    ctx_t[j],
                                    va_sb[i - 1][:, mq : mq + 17],
                                    ex_tiles[i - 1][:, 512 * j : 512 * (j + 1)],
                                    start=(i - 1 == 0),
                                    stop=False,
                                )
                    for j in range(NHQ):
                        mq = 17 * (NHQ * t + j)
                        nc.tensor.matmul(
                            ctx_t[j],
                            va_sb[NS - 1][:, mq : mq + 17],
                            ex_tiles[NS - 1][:, 512 * j : 512 * (j + 1)],
                            start=False,
                            stop=True,
                        )

                    # stage + denominators
                    den = ap_.tile([NHQ, 512], F32, name=f"den{t}{n}", tag="den", bufs=2)
                    stages = []
                    for j in range(NHQ):
                        st = ap_.tile(
                            [17, 512], F32, name=f"st{t}{n}{j}", tag="stage", bufs=8
                        )
                        nc.vector.tensor_copy(st, ctx_t[j])
                        stages.append(st)
                        nc.sync.dma_start(out=den[j : j + 1, :], in_=st[16:17, :])
                    with tc.high_priority(offset=-160):
                        recip = ap_.tile(
                            [NHQ, 512], F32, name=f"rc{t}{n}", tag="recip", bufs=2
                        )
                        scratch = ap_.tile(
                            [NHQ, 512], F32, name=f"rs{t}{n}", tag="recip", bufs=2
                        )
                        nc.vector.reciprocal_approx_accurate(recip, den, scratch)
                        recipr = ap_.tile(
                            [NHQ, 512], F32R, name=f"rr{t}{n}", tag="recipr", bufs=1
                        )
                        nc.vector.tensor_copy(recipr, recip)
                        rbw = ps.tile(
                            [128, 512], F32, name=f"rbp{t}{n}", tag="ctx", bufs=4
                        )
                        nc.tensor.matmul(rbw, ind_sb, recipr, start=True, stop=True)
                        for j in range(NHQ):
                            if False:
                                rb = ap_.tile(
                                    [16, 512], F32, name=f"rb{t}{n}{j}", tag="rb", bufs=6
                                )
                                nc.vector.tensor_copy(rb, rbw[32 * j : 32 * j + 16, :])
                                nc.vector.tensor_mul(
                                    ctxp[t][32 * j : 32 * j + 16, 512 * n : 512 * (n + 1)],
                                    stages[j][0:16, :],
                                    rb,
                                )
                            else:
                                nc.vector.scalar_tensor_tensor(
                                    ctxp[t][32 * j : 32 * j + 16, 512 * n : 512 * (n + 1)],
                                    rbw[32 * j : 32 * j + 16, :],
                                    1.0,
                                    stages[j][0:16, :],
                                    mybir.AluOpType.mult,
                                    mybir.AluOpType.mult,
                                )

                # n=1: remaining output projection (n=0's groups were
                # interleaved into this half's i-loops above)
                if phases >= 3 and n == 1:
                    for m in range(4, 8):
                        for dc in range(2):
                            outproj_group(m, dc)

    nc.finalize()
    return nc


def prep_core_weights(g, Wq, bq, Wk, bk, Wv, bv, Wo):
    C0 = 256 * g
    wqt = np.zeros((D, 512), np.float32)
    wkt = np.zeros((D, 512), np.float32)
    wvt = np.zeros((D, VW), np.float32)
    wot = np.zeros((512, D), np.float32)
    bvrow = np.zeros((1, VW), np.float32)
    bqp = np.zeros((128, NQUAD), np.float32)
    bkp = np.zeros((128, NQUAD), np.float32)
    for t in range(NQUAD):
        for j in range(NHQ):
            src = C0 + 64 * t + 16 * j
            wqt[:, 128 * t + 32 * j : 128 * t + 32 * j + E] = Wq[src : src + E, :].T
            wkt[:, 128 * t + 32 * j : 128 * t + 32 * j + E] = Wk[src : src + E, :].T
            m = NHQ * t + j
            wvt[:, 17 * m : 17 * m + E] = Wv[src : src + E, :].T
            wot[128 * t + 32 * j : 128 * t + 32 * j + E, :] = Wo[:, src : src + E].T
            bvrow[0, 17 * m : 17 * m + E] = bv[src : src + E]
            bvrow[0, 17 * m + E] = 1.0
            bqp[32 * j : 32 * j + E, t] = bq[src : src + E]
            bkp[32 * j : 32 * j + E, t] = bk[src : src + E]
    ind = np.zeros((NHQ, 128), np.float32)
    for j in range(NHQ):
        ind[j, 32 * j : 32 * j + E] = 1.0
    return {
        "wqt": round_fp32r(wqt),
        "wkt": round_fp32r(wkt),
        "wvt": round_fp32r(wvt),
        "wot": round_fp32r(wot),
        "bvrow": bvrow,
        "bqp": bqp,
        "bkp": bkp,
        "ind": round_fp32r(ind),
    }


def prep_in_maps(Q, K, V, Wq, bq, Wk, bk, Wv, bv, Wo):
    group_w = [prep_core_weights(g, Wq, bq, Wk, bk, Wv, bv, Wo) for g in range(4)]
    xt = []
    for b in range(B):
        xt.append(
            {
                "xt_q": round_fp32r(np.ascontiguousarray(Q[b].T)),
                "xt_k": round_fp32r(np.ascontiguousarray(K[b].T)),
                "xt_v": round_fp32r(np.ascontiguousarray(V[b].T)),
            }
        )
    in_maps = []
    for c in range(8):
        b, g = c // 4, c % 4
        m = dict(group_w[g])
        m.update(xt[b])
        in_maps.append(m)
    return in_maps


def assemble_output(results, bo):
    out = np.zeros((B, S, D), np.float32)
    for b in range(B):
        acc = np.zeros((S, D), np.float64)
        for g in range(4):
            acc += results[4 * b + g]["out_part"].astype(np.float64)
        out[b] = (acc + bo.astype(np.float64)).astype(np.float32)
    return out


_NC_CACHE = {}


def _get_nc():
    if "nc" not in _NC_CACHE:
        _NC_CACHE["nc"] = build_nc()
    return _NC_CACHE["nc"]


def kernel(Q, K, V, Wq, bq, Wk, bk, Wv, bv, Wo, bo):
    import time

    from concourse.bass_utils import run_bass_kernel_spmd

    nc = _get_nc()
    in_maps = prep_in_maps(
        np.asarray(Q, np.float32),
        np.asarray(K, np.float32),
        np.asarray(V, np.float32),
        np.asarray(Wq, np.float32),
        np.asarray(bq, np.float32),
        np.asarray(Wk, np.float32),
        np.asarray(bk, np.float32),
        np.asarray(Wv, np.float32),
        np.asarray(bv, np.float32),
        np.asarray(Wo, np.float32),
    )
    # Retries: a first execution after NEFF load occasionally hits a
    # transient NRT_EXEC_UNIT_UNRECOVERABLE; re-running recovers.
    last = None
    for attempt in range(3):
        try:
            res = run_bass_kernel_spmd(nc, in_maps, list(range(8)))
            return assemble_output(res.results, np.asarray(bo, np.float32))
        except Exception as e:
            last = e
            time.sleep(3)
    raise last

